# revision 14
# baseline (speedup 1.0000x reference)
"""TBCNN conv-node kernel for Trainium2 (8 NeuronCores, batch-sharded).

Math (derived from the reference, including its faithful-reshape quirk):
  out[b,n,o] = tanh( nodes[b,n,:] @ Wt + Sr[b,n,:] @ Wr + Sl[b,n,:] @ Wl + bias[o] )
    Sr[b,n,:] = sum_c cr[b,n,c] * nodes[b, ch[b,n,c], :]
    Sl[b,n,:] = sum_c cl[b,n,c] * nodes[b, ch[b,n,c], :]
  where Wt/Wr/Wl are rows 0::3 / 1::3 / 2::3 of concat([w_t, w_r, w_l]) (the
  reference reshapes [F,3] -> [3,F] raw), and cr/cl are the eta_r/eta_l
  coefficients, both forced to 0 where ch==0 so the zero-row lookup semantics
  hold while gathering from the raw nodes table.

v2 design (vs. the fp32 baseline):
  - All gather/matmul traffic in fp16: halves the random-row HBM gather bytes
    (256B rows) and cuts PE LDWEIGHTS from 512 to 128 cycles per stationary.
  - eta coefficients are computed on host directly into the block-diagonal
    rhs layout (aall16), eliminating the on-device coefficient pipeline and
    its PE transposes.
  - nodesT is host-pretransposed and DMA'd straight to SBUF.
  - Stage 2 is reoriented to out[n(part), o(free)] = lhsT(nodesT/srt/slt
    feature-major tiles) x rhs(W [F,O]); output DMAs out in natural layout,
    eliminating all output transposes. Bias is folded in via a K=1 matmul
    (ones row x bias row) that also start=True-initializes the PSUM bank.
"""

import numpy as np
from functools import lru_cache

B, N, C, F, O = 16, 2048, 16, 128, 128
NCORES = 8
BPC = B // NCORES  # batches per core
KBLK = 8  # 8-node gather blocks per chunk (KBLK*128 rows per dma_gather)
NBLK = N // 8  # 256 blocks per batch
NCHUNK = NBLK // KBLK  # chunks per batch
NPC = KBLK * 8  # nodes covered per chunk
RNDCH = 128 // NPC  # chunks per stage-2 round (128 nodes each)
GBUFS = 6  # gather tile double-buffering depth
DSCRATCH = 16384  # SWDGE descriptor carveout bytes/partition (default)


@lru_cache(maxsize=1)
def _build():
    import concourse.bass as bass
    import concourse.bacc as bacc
    import concourse.tile as tile
    from concourse import mybir

    f16 = mybir.dt.float16
    f32 = mybir.dt.float32
    i16 = mybir.dt.int16
    Act = mybir.ActivationFunctionType

    nc = bacc.Bacc("TRN2", target_bir_lowering=False, debug=False,
                   num_devices=NCORES, num_swdge_queues=4,
                   dynamic_dma_scratch_size=DSCRATCH)

    nodes16_d = nc.dram_tensor("nodes16", [BPC, N, F], f16, kind="ExternalInput")
    nodesT_d = nc.dram_tensor("nodesT16", [BPC, 128, N], f16, kind="ExternalInput")
    cht_d = nc.dram_tensor("cht", [BPC, 128, N], i16, kind="ExternalInput")
    aall_d = nc.dram_tensor("aall16", [BPC, 128, NBLK * 16], f16, kind="ExternalInput")
    w3_d = nc.dram_tensor("w3c16", [F, 3 * O], f16, kind="ExternalInput")
    bo_d = nc.dram_tensor("bo16", [1, 256], f16, kind="ExternalInput")
    out_d = nc.dram_tensor("out16", [BPC, N, O], f16, kind="ExternalOutput")

    with tile.TileContext(nc) as tc:
        with (
            tc.tile_pool(name="const", bufs=1) as cpool,
            tc.tile_pool(name="work", bufs=2) as pool,
            tc.tile_pool(name="gath", bufs=GBUFS) as gpool,
            tc.tile_pool(name="perb", bufs=2) as ppool,
            tc.tile_pool(name="ps1", bufs=4, space="PSUM") as ps1pool,
            tc.tile_pool(name="ps2", bufs=2, space="PSUM") as ps2pool,
        ):
            # ---- batch-0 gather indices first: they gate the first gather
            cht0 = ppool.tile([128, N], i16)
            nc.sync.dma_start(cht0[:], cht_d.ap()[0])

            # ---------------- constants ----------------
            w3_s = cpool.tile([F, 3 * O], f16)
            bo_s = cpool.tile([1, 256], f16)
            nc.sync.dma_start(w3_s[:], w3_d.ap())
            nc.sync.dma_start(bo_s[:], bo_d.ap())
            wt_s = w3_s[:, 0:O]
            wr_s = w3_s[:, O:2 * O]
            wl_s = w3_s[:, 2 * O:3 * O]
            brow_s = bo_s[:, 0:128]
            ones_s = bo_s[:, 128:256]

            for b in range(BPC):
                # ------------- per-batch inputs -------------
                if b == 0:
                    cht = cht0
                else:
                    cht = ppool.tile([128, N], i16)
                    nc.sync.dma_start(cht[:], cht_d.ap()[b])
                aall = ppool.tile([128, NBLK * 16], f16)
                nc.sync.dma_start(aall[:], aall_d.ap()[b])
                nodesT = ppool.tile([128, N], f16)
                nc.sync.dma_start(nodesT[:], nodesT_d.ap()[b])

                # ------------- gather + stage 1 + interleaved stage 2 -------
                srt = ppool.tile([128, N], f16)
                slt = ppool.tile([128, N], f16)
                for q in range(NCHUNK):
                    g = gpool.tile([128, KBLK * 128], f16)
                    nc.gpsimd.dma_gather(
                        out_ap=g[:].rearrange("p (g f) -> p g f", f=128),
                        in_ap=nodes16_d.ap()[b],
                        idxs_ap=cht[:, q * NPC:(q + 1) * NPC],
                        num_idxs=KBLK * 128,
                        num_idxs_reg=KBLK * 128,
                        elem_size=128,
                        queue_num=(b * NCHUNK + q) % 4,
                    )
                    ps1 = ps1pool.tile([128, KBLK * 16], f32)
                    for gl in range(KBLK):
                        blk = q * KBLK + gl
                        nc.tensor.matmul(
                            ps1[:, gl * 16:(gl + 1) * 16],
                            lhsT=g[:, gl * 128:(gl + 1) * 128],
                            rhs=aall[:, blk * 16:(blk + 1) * 16],
                            start=True,
                            stop=True,
                        )
                    nc.vector.tensor_copy(
                        srt[:, q * NPC:(q + 1) * NPC].rearrange(
                            "p (g m) -> p g m", m=8
                        ),
                        ps1[:].rearrange("p (g m) -> p g m", m=16)[:, :, 0:8],
                    )
                    nc.vector.tensor_copy(
                        slt[:, q * NPC:(q + 1) * NPC].rearrange(
                            "p (g m) -> p g m", m=8
                        ),
                        ps1[:].rearrange("p (g m) -> p g m", m=16)[:, :, 8:16],
                    )

                    # ---- stage 2 for the 128 nodes completed by this chunk:
                    # out[n(part), o] = nodesT/srt/slt tile x Wt/Wr/Wl, bias
                    # seeded via a K=1 matmul (ones row x bias row).
                    if q % RNDCH == RNDCH - 1:
                        t = q // RNDCH
                        sl = slice(t * 128, (t + 1) * 128)
                        ps2 = ps2pool.tile([128, 128], f32)
                        nc.tensor.matmul(
                            ps2[:], lhsT=ones_s[:], rhs=brow_s[:],
                            start=True, stop=False, skip_group_check=True,
                        )
                        nc.tensor.matmul(
                            ps2[:], lhsT=nodesT[:, sl], rhs=wt_s[:],
                            start=False, stop=False, skip_group_check=True,
                        )
                        nc.tensor.matmul(
                            ps2[:], lhsT=srt[:, sl], rhs=wr_s[:],
                            start=False, stop=False, skip_group_check=True,
                        )
                        nc.tensor.matmul(
                            ps2[:], lhsT=slt[:, sl], rhs=wl_s[:],
                            start=False, stop=True, skip_group_check=True,
                        )
                        ot = pool.tile([128, 128], f16)
                        nc.scalar.activation(ot[:], ps2[:], Act.Tanh)
                        nc.sync.dma_start(
                            out_d.ap()[b, t * 128:(t + 1) * 128, :], ot[:]
                        )

    nc.compile()
    return nc


def _host_prep(nodes, children, w_t, w_r, w_l, b_conv):
    nodes = np.asarray(nodes, dtype=np.float32)
    children = np.asarray(children, dtype=np.int32)
    w_t = np.asarray(w_t, dtype=np.float32)
    w_r = np.asarray(w_r, dtype=np.float32)
    w_l = np.asarray(w_l, dtype=np.float32)
    b_conv = np.asarray(b_conv, dtype=np.float32)

    wflat = np.concatenate([w_t, w_r, w_l], axis=0)  # [3F, O]
    w3 = np.ascontiguousarray(
        np.concatenate([wflat[0::3], wflat[1::3], wflat[2::3]], axis=1)
    ).astype(np.float16)  # [F, 3O] = [Wt | Wr | Wl]
    bo = np.concatenate(
        [b_conv.astype(np.float16), np.ones((128,), dtype=np.float16)]
    )[None, :]  # [1, 256] = [bias row | ones row]

    nodes16 = nodes.astype(np.float16)  # [B, N, F]
    nodesT16 = np.ascontiguousarray(nodes16.transpose(0, 2, 1))  # [B, F, N]

    # eta coefficients (host, fp32 then cast), forced 0 where ch==0
    mask = (children != 0).astype(np.float32)  # [B, N, C]
    nsib = mask.sum(axis=2)  # [B, N]
    denom = nsib - 1.0
    safe = np.where(denom == 0.0, 1.0, denom)
    crg = (np.arange(C, dtype=np.float32)[None, None, :] * mask) / safe[:, :, None]
    k0row = np.zeros((C,), dtype=np.float32)
    k0row[0] = 0.5
    cr = np.where((nsib == 1.0)[:, :, None], k0row[None, None, :], crg)
    creff = (cr * mask).astype(np.float16)  # [B, N, C]
    cleff = (mask.astype(np.float16) - creff)  # [B, N, C]

    # block-diagonal stage-1 rhs: aall[b, p, blk*16+j]
    #   p = m*16 + c (node-in-block m, child c); j<8 -> Sr col m==j, j>=8 -> Sl
    cre_b = creff.reshape(B, NBLK, 8, C).transpose(0, 2, 3, 1)  # [B, m, c, blk]
    cle_b = cleff.reshape(B, NBLK, 8, C).transpose(0, 2, 3, 1)
    aallv = np.zeros((B, 8, C, NBLK, 16), dtype=np.float16)
    for m in range(8):
        aallv[:, m, :, :, m] = cre_b[:, m]
        aallv[:, m, :, :, 8 + m] = cle_b[:, m]
    aall = aallv.reshape(B, 128, NBLK * 16)

    cht_all = np.tile(
        children.transpose(0, 2, 1).astype(np.int16), (1, 8, 1)
    )  # [B, 128, N]

    in_maps = []
    for core in range(NCORES):
        bs = slice(core * BPC, (core + 1) * BPC)
        in_maps.append(
            {
                "nodes16": np.ascontiguousarray(nodes16[bs]),
                "nodesT16": np.ascontiguousarray(nodesT16[bs]),
                "cht": np.ascontiguousarray(cht_all[bs]),
                "aall16": np.ascontiguousarray(aall[bs]),
                "w3c16": w3,
                "bo16": bo,
            }
        )
    return in_maps


def _run(inputs, trace=False):
    from concourse.bass_utils import run_bass_kernel_spmd

    nc = _build()
    in_maps = _host_prep(
        inputs["nodes"], inputs["children"], inputs["w_t"], inputs["w_r"],
        inputs["w_l"], inputs["b_conv"],
    )
    res = run_bass_kernel_spmd(nc, in_maps, list(range(NCORES)), trace=trace)
    out = np.concatenate([r["out16"] for r in res.results], axis=0)
    return out.astype(np.float32), res


def kernel(nodes, children, feature_size=None, w_t=None, w_r=None, w_l=None,
           b_conv=None, **_unused):
    out, _ = _run(
        {
            "nodes": nodes,
            "children": children,
            "w_t": w_t,
            "w_r": w_r,
            "w_l": w_l,
            "b_conv": b_conv,
        }
    )
    return out


# revision 18
# speedup vs baseline: 1.0162x; 1.0162x over previous
"""TBCNN conv-node kernel for Trainium2 (8 NeuronCores, batch-sharded).

Math (derived from the reference, including its faithful-reshape quirk):
  out[b,n,o] = tanh( nodes[b,n,:] @ Wt + Sr[b,n,:] @ Wr + Sl[b,n,:] @ Wl + bias[o] )
    Sr[b,n,:] = sum_c cr[b,n,c] * nodes[b, ch[b,n,c], :]
    Sl[b,n,:] = sum_c cl[b,n,c] * nodes[b, ch[b,n,c], :]
  where Wt/Wr/Wl are rows 0::3 / 1::3 / 2::3 of concat([w_t, w_r, w_l]) (the
  reference reshapes [F,3] -> [3,F] raw), and cr/cl are the eta_r/eta_l
  coefficients, both forced to 0 where ch==0 so the zero-row lookup semantics
  hold while gathering from the raw nodes table.

v2 design (vs. the fp32 baseline):
  - All gather/matmul traffic in fp16: halves the random-row HBM gather bytes
    (256B rows) and cuts PE LDWEIGHTS from 512 to 128 cycles per stationary.
  - eta coefficients are computed on host directly into the block-diagonal
    rhs layout (aall16), eliminating the on-device coefficient pipeline and
    its PE transposes.
  - nodesT is host-pretransposed and DMA'd straight to SBUF.
  - Stage 2 is reoriented to out[n(part), o(free)] = lhsT(nodesT/srt/slt
    feature-major tiles) x rhs(W [F,O]); output DMAs out in natural layout,
    eliminating all output transposes. Bias is folded in via a K=1 matmul
    (ones row x bias row) that also start=True-initializes the PSUM bank.
"""

import numpy as np
from functools import lru_cache

B, N, C, F, O = 16, 2048, 16, 128, 128
NCORES = 8
BPC = B // NCORES  # batches per core
KBLK = 8  # 8-node gather blocks per chunk (KBLK*128 rows per dma_gather)
NBLK = N // 8  # 256 blocks per batch
NCHUNK = NBLK // KBLK  # chunks per batch
NPC = KBLK * 8  # nodes covered per chunk
RNDCH = 128 // NPC  # chunks per stage-2 round (128 nodes each)
GBUFS = 8  # gather tile double-buffering depth
DSCRATCH = 16384  # SWDGE descriptor carveout bytes/partition (default)


@lru_cache(maxsize=1)
def _build():
    import concourse.bass as bass
    import concourse.bacc as bacc
    import concourse.tile as tile
    from concourse import mybir

    f16 = mybir.dt.float16
    f32 = mybir.dt.float32
    i16 = mybir.dt.int16
    Act = mybir.ActivationFunctionType

    nc = bacc.Bacc("TRN2", target_bir_lowering=False, debug=False,
                   num_devices=NCORES, num_swdge_queues=4,
                   dynamic_dma_scratch_size=DSCRATCH)

    nodes16_d = nc.dram_tensor("nodes16", [BPC, N, F], f16, kind="ExternalInput")
    nodesT_d = nc.dram_tensor("nodesT16", [BPC, 128, N], f16, kind="ExternalInput")
    cht_d = nc.dram_tensor("cht", [BPC, 128, N], i16, kind="ExternalInput")
    aall_d = nc.dram_tensor("aall16", [BPC, 128, NBLK * 16], f16, kind="ExternalInput")
    w3_d = nc.dram_tensor("w3c16", [F, 3 * O], f16, kind="ExternalInput")
    bo_d = nc.dram_tensor("bo16", [1, 256], f16, kind="ExternalInput")
    out_d = nc.dram_tensor("out16", [BPC, N, O], f16, kind="ExternalOutput")

    with tile.TileContext(nc) as tc:
        with (
            tc.tile_pool(name="const", bufs=1) as cpool,
            tc.tile_pool(name="work", bufs=2) as pool,
            tc.tile_pool(name="gath", bufs=GBUFS) as gpool,
            tc.tile_pool(name="perb", bufs=2) as ppool,
            tc.tile_pool(name="ps1", bufs=6, space="PSUM") as ps1pool,
            tc.tile_pool(name="ps2", bufs=2, space="PSUM") as ps2pool,
        ):
            # ---- batch-0 gather indices first: they gate the first gather
            cht0 = ppool.tile([128, N], i16)
            nc.sync.dma_start(cht0[:], cht_d.ap()[0])

            # ---------------- constants ----------------
            w3_s = cpool.tile([F, 3 * O], f16)
            bo_s = cpool.tile([1, 256], f16)
            nc.sync.dma_start(w3_s[:], w3_d.ap())
            nc.sync.dma_start(bo_s[:], bo_d.ap())
            wt_s = w3_s[:, 0:O]
            wr_s = w3_s[:, O:2 * O]
            wl_s = w3_s[:, 2 * O:3 * O]
            brow_s = bo_s[:, 0:128]
            ones_s = bo_s[:, 128:256]

            nidx_reg = nc.gpsimd.to_reg(KBLK * 128)

            for b in range(BPC):
                # ------------- per-batch inputs -------------
                if b == 0:
                    cht = cht0
                else:
                    cht = ppool.tile([128, N], i16)
                    nc.sync.dma_start(cht[:], cht_d.ap()[b])
                aall = ppool.tile([128, NBLK * 16], f16)
                nc.sync.dma_start(aall[:], aall_d.ap()[b])
                nodesT = ppool.tile([128, N], f16)
                nc.sync.dma_start(nodesT[:], nodesT_d.ap()[b])

                # ------------- gather + stage 1 + interleaved stage 2 -------
                srt = ppool.tile([128, N], f16)
                slt = ppool.tile([128, N], f16)
                for q in range(NCHUNK):
                    g = gpool.tile([128, KBLK * 128], f16)
                    nc.gpsimd.dma_gather(
                        out_ap=g[:].rearrange("p (g f) -> p g f", f=128),
                        in_ap=nodes16_d.ap()[b],
                        idxs_ap=cht[:, q * NPC:(q + 1) * NPC],
                        num_idxs=KBLK * 128,
                        num_idxs_reg=nidx_reg,
                        elem_size=128,
                        queue_num=(b * NCHUNK + q) % 4,
                    )
                    ps1 = ps1pool.tile([128, KBLK * 16], f32)
                    for gl in range(KBLK):
                        blk = q * KBLK + gl
                        nc.tensor.matmul(
                            ps1[:, gl * 16:(gl + 1) * 16],
                            lhsT=g[:, gl * 128:(gl + 1) * 128],
                            rhs=aall[:, blk * 16:(blk + 1) * 16],
                            start=True,
                            stop=True,
                        )
                    nc.vector.tensor_copy(
                        srt[:, q * NPC:(q + 1) * NPC].rearrange(
                            "p (g m) -> p g m", m=8
                        ),
                        ps1[:].rearrange("p (g m) -> p g m", m=16)[:, :, 0:8],
                    )
                    nc.vector.tensor_copy(
                        slt[:, q * NPC:(q + 1) * NPC].rearrange(
                            "p (g m) -> p g m", m=8
                        ),
                        ps1[:].rearrange("p (g m) -> p g m", m=16)[:, :, 8:16],
                    )

                    # ---- stage 2 for the 128 nodes completed by this chunk:
                    # out[n(part), o] = nodesT/srt/slt tile x Wt/Wr/Wl, bias
                    # seeded via a K=1 matmul (ones row x bias row).
                    if q % RNDCH == RNDCH - 1:
                        t = q // RNDCH
                        sl = slice(t * 128, (t + 1) * 128)
                        ps2 = ps2pool.tile([128, 128], f32)
                        nc.tensor.matmul(
                            ps2[:], lhsT=ones_s[:], rhs=brow_s[:],
                            start=True, stop=False, skip_group_check=True,
                        )
                        nc.tensor.matmul(
                            ps2[:], lhsT=nodesT[:, sl], rhs=wt_s[:],
                            start=False, stop=False, skip_group_check=True,
                        )
                        nc.tensor.matmul(
                            ps2[:], lhsT=srt[:, sl], rhs=wr_s[:],
                            start=False, stop=False, skip_group_check=True,
                        )
                        nc.tensor.matmul(
                            ps2[:], lhsT=slt[:, sl], rhs=wl_s[:],
                            start=False, stop=True, skip_group_check=True,
                        )
                        ot = pool.tile([128, 128], f16)
                        nc.scalar.activation(ot[:], ps2[:], Act.Tanh)
                        nc.sync.dma_start(
                            out_d.ap()[b, t * 128:(t + 1) * 128, :], ot[:]
                        )

    nc.compile()
    return nc


def _host_prep(nodes, children, w_t, w_r, w_l, b_conv):
    nodes = np.asarray(nodes, dtype=np.float32)
    children = np.asarray(children, dtype=np.int32)
    w_t = np.asarray(w_t, dtype=np.float32)
    w_r = np.asarray(w_r, dtype=np.float32)
    w_l = np.asarray(w_l, dtype=np.float32)
    b_conv = np.asarray(b_conv, dtype=np.float32)

    wflat = np.concatenate([w_t, w_r, w_l], axis=0)  # [3F, O]
    w3 = np.ascontiguousarray(
        np.concatenate([wflat[0::3], wflat[1::3], wflat[2::3]], axis=1)
    ).astype(np.float16)  # [F, 3O] = [Wt | Wr | Wl]
    bo = np.concatenate(
        [b_conv.astype(np.float16), np.ones((128,), dtype=np.float16)]
    )[None, :]  # [1, 256] = [bias row | ones row]

    nodes16 = nodes.astype(np.float16)  # [B, N, F]
    nodesT16 = np.ascontiguousarray(nodes16.transpose(0, 2, 1))  # [B, F, N]

    # eta coefficients (host, fp32 then cast), forced 0 where ch==0
    mask = (children != 0).astype(np.float32)  # [B, N, C]
    nsib = mask.sum(axis=2)  # [B, N]
    denom = nsib - 1.0
    safe = np.where(denom == 0.0, 1.0, denom)
    crg = (np.arange(C, dtype=np.float32)[None, None, :] * mask) / safe[:, :, None]
    k0row = np.zeros((C,), dtype=np.float32)
    k0row[0] = 0.5
    cr = np.where((nsib == 1.0)[:, :, None], k0row[None, None, :], crg)
    creff = (cr * mask).astype(np.float16)  # [B, N, C]
    cleff = (mask.astype(np.float16) - creff)  # [B, N, C]

    # block-diagonal stage-1 rhs: aall[b, p, blk*16+j]
    #   p = m*16 + c (node-in-block m, child c); j<8 -> Sr col m==j, j>=8 -> Sl
    cre_b = creff.reshape(B, NBLK, 8, C).transpose(0, 2, 3, 1)  # [B, m, c, blk]
    cle_b = cleff.reshape(B, NBLK, 8, C).transpose(0, 2, 3, 1)
    aallv = np.zeros((B, 8, C, NBLK, 16), dtype=np.float16)
    for m in range(8):
        aallv[:, m, :, :, m] = cre_b[:, m]
        aallv[:, m, :, :, 8 + m] = cle_b[:, m]
    aall = aallv.reshape(B, 128, NBLK * 16)

    cht_all = np.tile(
        children.transpose(0, 2, 1).astype(np.int16), (1, 8, 1)
    )  # [B, 128, N]

    in_maps = []
    for core in range(NCORES):
        bs = slice(core * BPC, (core + 1) * BPC)
        in_maps.append(
            {
                "nodes16": np.ascontiguousarray(nodes16[bs]),
                "nodesT16": np.ascontiguousarray(nodesT16[bs]),
                "cht": np.ascontiguousarray(cht_all[bs]),
                "aall16": np.ascontiguousarray(aall[bs]),
                "w3c16": w3,
                "bo16": bo,
            }
        )
    return in_maps


def _run(inputs, trace=False):
    from concourse.bass_utils import run_bass_kernel_spmd

    nc = _build()
    in_maps = _host_prep(
        inputs["nodes"], inputs["children"], inputs["w_t"], inputs["w_r"],
        inputs["w_l"], inputs["b_conv"],
    )
    res = run_bass_kernel_spmd(nc, in_maps, list(range(NCORES)), trace=trace)
    out = np.concatenate([r["out16"] for r in res.results], axis=0)
    return out.astype(np.float32), res


def kernel(nodes, children, feature_size=None, w_t=None, w_r=None, w_l=None,
           b_conv=None, **_unused):
    out, _ = _run(
        {
            "nodes": nodes,
            "children": children,
            "w_t": w_t,
            "w_r": w_r,
            "w_l": w_l,
            "b_conv": b_conv,
        }
    )
    return out
